# revision 13
# baseline (speedup 1.0000x reference)
"""Trainium2 Bass kernel: DragonFly sparsity plugin (topk_masking).

Reference semantics (per batch sample, fp32):
  low  = x[:576].reshape(24, 24, 1024)   -> l2-normalize last dim
  high = x[576:].reshape(24, 96, 1024)   -> l2-normalize last dim
  q    = low_hat.mean(axis=1)            # [24, 1024]
  inner= einsum('pd,pgd->pg', q, high_hat)
  idx  = top_k(inner, 8)                 # [24, 8]
  out  = concat(low_hat.reshape(576, d), high_hat[p, idx].reshape(192, d))

Sharding: pure data parallel, 2 batch samples per core x 8 cores.
"""

import numpy as np

import bass_rust
import concourse.bass as bass
import concourse.tile as tile
from concourse import mybir
from concourse.bass import IndirectOffsetOnAxis
from concourse.bass_utils import run_bass_kernel_spmd


def _patch_tile_drain():
    """The walrus build in this image rejects instructions carrying >2 sync
    waits (CoreV3 setupSyncWait: "Too many sync wait commands"). Tile's
    end-of-kernel drain attaches one wait per live semaphore, so spread the
    waits over single-wait NOP carriers ahead of the drain instead."""
    if getattr(tile.TileContext, "_drain_patch_installed", False):
        return

    def patched(self, tick_clock, wait_clock):
        nc = self.nc
        probe = nc.sync.nop(nofuse=True)
        wait_clock.add_sem_waits(
            probe.ins, tile.ScopedClock({None: tick_clock.global_clock})
        )
        si = probe.ins.sync_info
        waits = list(si.on_wait) if si is not None else []
        if si is not None:
            si.on_wait = waits[:1]
        for i in range(1, len(waits)):
            n = nc.sync.nop(nofuse=True)
            n.ins.sync_info = bass_rust.SyncInfo(on_wait=[waits[i]], on_update=[])
        nc.sync.drain()
        nc.all_engine_barrier()
        popped = nc._tile_sem_poison_stack.pop()
        assert popped is self._sem_poison
        nc.clear_and_free_semaphores(list(self.sems.allocated().values()))
        nc.all_engine_barrier()

    tile.TileContext._drain_and_barrier = patched
    tile.TileContext._drain_patch_installed = True


_patch_tile_drain()

MAX_SYNC_WAITS = 2


def _split_excess_waits(nc, max_waits=MAX_SYNC_WAITS):
    """Walrus in this image caps sync waits per instruction; hoist excess
    waits onto single-wait NOPs queued just before the instruction on the
    same engine (identical blocking semantics)."""
    k = 0
    for f in nc.m.functions:
        for b in f.blocks:
            rewritten = []
            dirty = False
            for ins in b.instructions:
                si = ins.sync_info
                waits = list(si.on_wait) if si is not None else []
                n_upd = len(si.on_update) if si is not None else 0
                budget = max(max_waits - n_upd, 1 if waits else 0)
                if len(waits) > budget:
                    dirty = True
                    n_extra = len(waits) - budget
                    for j in range(n_extra):
                        n = mybir.InstNoOp(
                            name=f"I-wsplit-{k}", ins=[], outs=[], engine=ins.engine
                        )
                        k += 1
                        n.sync_info = bass_rust.SyncInfo(
                            on_wait=[waits[j]], on_update=[]
                        )
                        rewritten.append(n)
                    si.on_wait = waits[n_extra:]
                rewritten.append(ins)
            if dirty:
                b.instructions = rewritten

BSZ, SEQ, D = 16, 2880, 1024
N_LOW, N_HIGH = 576, 2304
P_PATCH = 24  # patches per sample
GL, GH = 24, 96  # low/high tokens per patch
TOP_K = 8
N_CORES = 8
SPC = BSZ // N_CORES  # samples per core
OUT_SEQ = N_LOW + P_PATCH * TOP_K  # 768

F32 = mybir.dt.float32
U32 = mybir.dt.uint32
AF = mybir.ActivationFunctionType
OP = mybir.AluOpType


def host_constants():
    # gmat[i, t, p] = 1/24 if low token t*128+i belongs to patch p else 0
    g = np.zeros((128, 5, P_PATCH), np.float32)
    for t in range(5):
        for i in range(128):
            tok = t * 128 + i
            if tok < N_LOW:
                g[i, t, tok // GL] = 1.0 / GL
    # ohmat[k, p, m] = 1 if k == p: stationary one-hot so E_p.T @ q broadcasts
    # q's row p across 96 partitions (PE operands must start at partition 0)
    oh = np.zeros((P_PATCH, P_PATCH, GH), np.float32)
    for p in range(P_PATCH):
        oh[p, p, :] = 1.0
    pbase = (N_LOW + GH * np.arange(P_PATCH, dtype=np.float32)).reshape(P_PATCH, 1)
    id96 = np.eye(GH, dtype=np.float32)
    return {"gmat": g, "ohmat": oh, "pbase": pbase, "id96": id96}


def build_program(split_waits=True):
    nc = bass.Bass()
    x = nc.declare_dram_parameter("x", [SPC * SEQ, D], F32, isOutput=False)
    gmat = nc.declare_dram_parameter("gmat", [128, 5, P_PATCH], F32, isOutput=False)
    ohmat = nc.declare_dram_parameter("ohmat", [P_PATCH, P_PATCH, GH], F32, isOutput=False)
    pbase = nc.declare_dram_parameter("pbase", [P_PATCH, 1], F32, isOutput=False)
    id96 = nc.declare_dram_parameter("id96", [GH, GH], F32, isOutput=False)
    out = nc.declare_dram_parameter("out", [SPC * OUT_SEQ, D], F32, isOutput=True)
    idxd = nc.dram_tensor("idxd", [SPC, P_PATCH * TOP_K, 1], U32)

    with tile.TileContext(nc) as tc:
        with (
            tc.tile_pool(name="consts", bufs=1) as consts,
            tc.tile_pool(name="lowp", bufs=4) as lowp,
            tc.tile_pool(name="highp", bufs=4) as highp,
            tc.tile_pool(name="gathp", bufs=2) as gathp,
            tc.tile_pool(name="scr", bufs=1) as scr,
            tc.tile_pool(name="small", bufs=6) as small,
            tc.tile_pool(name="accs", bufs=2) as accs,
            tc.tile_pool(name="psq", bufs=1, space="PSUM") as psq,
            tc.tile_pool(name="psqx", bufs=2, space="PSUM") as psqx,
            tc.tile_pool(name="psit", bufs=2, space="PSUM") as psit,
        ):
            g_sb = consts.tile([128, 5, P_PATCH], F32)
            nc.sync.dma_start(g_sb[:], gmat[:])
            oh_sb = consts.tile([P_PATCH, P_PATCH, GH], F32)
            nc.sync.dma_start(oh_sb[:], ohmat[:])
            pbase_sb = consts.tile([P_PATCH, 1], F32)
            nc.sync.dma_start(pbase_sb[:], pbase[:])
            id_sb = consts.tile([GH, GH], F32)
            nc.sync.dma_start(id_sb[:], id96[:])

            scr_act = scr.tile([128, D], F32)  # ACT throwaway output
            scr_dve = scr.tile([GH, D], F32)  # DVE throwaway output

            for s in range(SPC):
                x0 = s * SEQ
                o0 = s * OUT_SEQ

                # ---------------- low phase ----------------
                psum_q = psq.tile([P_PATCH, D], F32)
                for t in range(5):
                    rows = min(128, N_LOW - t * 128)
                    lt = lowp.tile([128, D], F32)
                    nc.sync.dma_start(
                        lt[:rows], x[x0 + t * 128 : x0 + t * 128 + rows, :]
                    )
                    ss = small.tile([128, 1], F32)
                    nc.scalar.activation(
                        scr_act[:rows], lt[:rows], AF.Square, accum_out=ss[:rows]
                    )
                    nrm = small.tile([128, 1], F32)
                    nc.scalar.activation(nrm[:rows], ss[:rows], AF.Sqrt)
                    rn = small.tile([128, 1], F32)
                    nc.vector.reciprocal(rn[:rows], nrm[:rows])
                    nc.vector.tensor_scalar_mul(lt[:rows], lt[:rows], rn[:rows])
                    for h in range(2):
                        nc.tensor.matmul(
                            psum_q[:, h * 512 : (h + 1) * 512],
                            lhsT=g_sb[:rows, t, :],
                            rhs=lt[:rows, h * 512 : (h + 1) * 512],
                            start=(t == 0),
                            stop=(t == 4),
                        )
                    nc.sync.dma_start(
                        out[o0 + t * 128 : o0 + t * 128 + rows, :], lt[:rows]
                    )
                q_sb = accs.tile([P_PATCH, D], F32)
                nc.scalar.activation(q_sb[:], psum_q[:], AF.Copy)

                # ---------------- high phase ----------------
                inner = accs.tile([GH, P_PATCH], F32)
                for p in range(P_PATCH):
                    r0 = x0 + N_LOW + p * GH
                    ht = highp.tile([GH, D], F32)
                    nc.sync.dma_start(ht[:], x[r0 : r0 + GH, :])
                    pqx = psqx.tile([GH, D], F32)
                    for h in range(2):
                        nc.tensor.matmul(
                            pqx[:, h * 512 : (h + 1) * 512],
                            lhsT=oh_sb[:, p, :],
                            rhs=q_sb[:, h * 512 : (h + 1) * 512],
                            start=True,
                            stop=True,
                        )
                    ssh = small.tile([GH, 1], F32)
                    nc.scalar.activation(
                        scr_act[:GH], ht[:], AF.Square, accum_out=ssh[:]
                    )
                    nrh = small.tile([GH, 1], F32)
                    nc.scalar.activation(nrh[:], ssh[:], AF.Sqrt)
                    rnh = small.tile([GH, 1], F32)
                    nc.vector.reciprocal(rnh[:], nrh[:])
                    # inner[:, p] = sum_d (h * rnorm) * qexp  — fused dot
                    nc.vector.scalar_tensor_tensor(
                        out=scr_dve[:],
                        in0=ht[:],
                        scalar=rnh[:],
                        in1=pqx[:],
                        op0=OP.mult,
                        op1=OP.mult,
                        accum_out=inner[:, p : p + 1],
                    )

                pit = psit.tile([P_PATCH, GH], F32)
                nc.tensor.transpose(pit[:], inner[:], id_sb[:])
                it_sb = accs.tile([P_PATCH, GH], F32)
                nc.scalar.activation(it_sb[:], pit[:], AF.Copy)

                mx8 = small.tile([P_PATCH, TOP_K], F32)
                nc.vector.max(out=mx8[:], in_=it_sb[:])
                ix8 = small.tile([P_PATCH, TOP_K], U32)
                nc.vector.max_index(out=ix8[:], in_max=mx8[:], in_values=it_sb[:])
                ixf = small.tile([P_PATCH, TOP_K], F32)
                nc.vector.tensor_copy(ixf[:], ix8[:])
                ixg = small.tile([P_PATCH, TOP_K], F32)
                nc.vector.tensor_scalar(
                    ixg[:],
                    ixf[:],
                    pbase_sb[:],
                    float(s * SEQ),
                    op0=OP.add,
                    op1=OP.add,
                )
                ixu = small.tile([P_PATCH, TOP_K], U32)
                nc.vector.tensor_copy(ixu[:], ixg[:])
                nc.sync.dma_start(
                    idxd[s].rearrange("(a b) c -> a (b c)", a=P_PATCH), ixu[:]
                )

                # ---------------- gather + renormalize ----------------
                for gi in range(2):
                    rows = 128 if gi == 0 else 64
                    base = gi * 128
                    ixcol = small.tile([128, 1], U32)
                    nc.sync.dma_start(ixcol[:rows], idxd[s, base : base + rows, :])
                    gt = gathp.tile([128, D], F32)
                    nc.gpsimd.indirect_dma_start(
                        out=gt[:rows],
                        out_offset=None,
                        in_=x[:],
                        in_offset=IndirectOffsetOnAxis(ap=ixcol[:rows], axis=0),
                    )
                    ssg = small.tile([128, 1], F32)
                    nc.scalar.activation(
                        scr_act[:rows], gt[:rows], AF.Square, accum_out=ssg[:rows]
                    )
                    nrg = small.tile([128, 1], F32)
                    nc.scalar.activation(nrg[:rows], ssg[:rows], AF.Sqrt)
                    rg = small.tile([128, 1], F32)
                    nc.vector.reciprocal(rg[:rows], nrg[:rows])
                    nc.vector.tensor_scalar_mul(gt[:rows], gt[:rows], rg[:rows])
                    nc.sync.dma_start(
                        out[o0 + N_LOW + base : o0 + N_LOW + base + rows, :],
                        gt[:rows],
                    )
    if split_waits:
        _split_excess_waits(nc)
    return nc


_CACHED = {}


def _get_program():
    if "nc" not in _CACHED:
        _CACHED["nc"] = build_program()
    return _CACHED["nc"]


def kernel(x: np.ndarray) -> np.ndarray:
    assert x.shape == (BSZ, SEQ, D), x.shape
    x = np.ascontiguousarray(x, dtype=np.float32)
    consts = host_constants()
    shards = x.reshape(N_CORES, SPC * SEQ, D)
    in_maps = [dict(consts, x=shards[i]) for i in range(N_CORES)]
    nc = _get_program()
    res = run_bass_kernel_spmd(nc, in_maps, core_ids=list(range(N_CORES)))
    outs = [res.results[i]["out"].reshape(SPC, OUT_SEQ, D) for i in range(N_CORES)]
    return np.concatenate(outs, axis=0).astype(np.float32)


# revision 20
# speedup vs baseline: 1.1314x; 1.1314x over previous
"""Trainium2 Bass kernel: DragonFly sparsity plugin (topk_masking).

Reference semantics (per batch sample, fp32):
  low  = x[:576].reshape(24, 24, 1024)   -> l2-normalize last dim
  high = x[576:].reshape(24, 96, 1024)   -> l2-normalize last dim
  q    = low_hat.mean(axis=1)            # [24, 1024]
  inner= einsum('pd,pgd->pg', q, high_hat)
  idx  = top_k(inner, 8)                 # [24, 8]
  out  = concat(low_hat.reshape(576, d), high_hat[p, idx].reshape(192, d))

Sharding: pure data parallel, 2 batch samples per core x 8 cores.
"""

import numpy as np

import bass_rust
import concourse.bacc as bacc
import concourse.bass as bass
import concourse.tile as tile
from concourse import mybir
from concourse.bass import IndirectOffsetOnAxis
from concourse.bass_utils import run_bass_kernel_spmd


def _patch_tile_drain():
    """The walrus build in this image rejects instructions carrying >2 sync
    waits (CoreV3 setupSyncWait: "Too many sync wait commands"). Tile's
    end-of-kernel drain attaches one wait per live semaphore, so spread the
    waits over single-wait NOP carriers ahead of the drain instead."""
    if getattr(tile.TileContext, "_drain_patch_installed", False):
        return

    def patched(self, tick_clock, wait_clock):
        nc = self.nc
        probe = nc.sync.nop(nofuse=True)
        wait_clock.add_sem_waits(
            probe.ins, tile.ScopedClock({None: tick_clock.global_clock})
        )
        si = probe.ins.sync_info
        waits = list(si.on_wait) if si is not None else []
        if si is not None:
            si.on_wait = waits[:1]
        for i in range(1, len(waits)):
            n = nc.sync.nop(nofuse=True)
            n.ins.sync_info = bass_rust.SyncInfo(on_wait=[waits[i]], on_update=[])
        nc.sync.drain()
        nc.all_engine_barrier()
        popped = nc._tile_sem_poison_stack.pop()
        assert popped is self._sem_poison
        nc.clear_and_free_semaphores(list(self.sems.allocated().values()))
        nc.all_engine_barrier()

    tile.TileContext._drain_and_barrier = patched
    tile.TileContext._drain_patch_installed = True


_patch_tile_drain()

MAX_SYNC_WAITS = 2


def _split_excess_waits(nc, max_waits=MAX_SYNC_WAITS):
    """Walrus in this image caps sync waits per instruction; hoist excess
    waits onto single-wait NOPs queued just before the instruction on the
    same engine (identical blocking semantics)."""
    k = 0
    for f in nc.m.functions:
        for b in f.blocks:
            rewritten = []
            dirty = False
            for ins in b.instructions:
                si = ins.sync_info
                waits = list(si.on_wait) if si is not None else []
                n_upd = len(si.on_update) if si is not None else 0
                budget = max(max_waits - n_upd, 1 if waits else 0)
                if len(waits) > budget:
                    dirty = True
                    n_extra = len(waits) - budget
                    for j in range(n_extra):
                        n = mybir.InstNoOp(
                            name=f"I-wsplit-{k}", ins=[], outs=[], engine=ins.engine
                        )
                        k += 1
                        n.sync_info = bass_rust.SyncInfo(
                            on_wait=[waits[j]], on_update=[]
                        )
                        rewritten.append(n)
                    si.on_wait = waits[n_extra:]
                rewritten.append(ins)
            if dirty:
                b.instructions = rewritten

BSZ, SEQ, D = 16, 2880, 1024
N_LOW, N_HIGH = 576, 2304
P_PATCH = 24  # patches per sample
GL, GH = 24, 96  # low/high tokens per patch
TOP_K = 8
N_CORES = 8
SPC = BSZ // N_CORES  # samples per core
OUT_SEQ = N_LOW + P_PATCH * TOP_K  # 768

F32 = mybir.dt.float32
U32 = mybir.dt.uint32
AF = mybir.ActivationFunctionType
OP = mybir.AluOpType


def host_constants():
    # gmat[i, t, p] = 1/24 if low token t*128+i belongs to patch p else 0
    g = np.zeros((128, 5, P_PATCH), np.float32)
    for t in range(5):
        for i in range(128):
            tok = t * 128 + i
            if tok < N_LOW:
                g[i, t, tok // GL] = 1.0 / GL
    pbase = (N_LOW + GH * np.arange(P_PATCH, dtype=np.float32)).reshape(P_PATCH, 1)
    id96 = np.eye(GH, dtype=np.float32)
    return {"gmat": g, "pbase": pbase, "id96": id96}


def build_program(split_waits=True):
    nc = bacc.Bacc()
    x = nc.declare_dram_parameter("x", [SPC * SEQ, D], F32, isOutput=False)
    gmat = nc.declare_dram_parameter("gmat", [128, 5, P_PATCH], F32, isOutput=False)
    pbase = nc.declare_dram_parameter("pbase", [P_PATCH, 1], F32, isOutput=False)
    id96 = nc.declare_dram_parameter("id96", [GH, GH], F32, isOutput=False)
    out = nc.declare_dram_parameter("out", [SPC * OUT_SEQ, D], F32, isOutput=True)
    idxd = nc.dram_tensor("idxd", [SPC, P_PATCH * TOP_K, 1], U32)

    with tile.TileContext(nc) as tc:
        with (
            tc.tile_pool(name="consts", bufs=1) as consts,
            tc.tile_pool(name="lowp", bufs=4) as lowp,
            tc.tile_pool(name="highp", bufs=4) as highp,
            tc.tile_pool(name="gathp", bufs=2) as gathp,
            tc.tile_pool(name="scr", bufs=1) as scr,
            tc.tile_pool(name="small", bufs=6) as small,
            tc.tile_pool(name="accs", bufs=2) as accs,
            tc.tile_pool(name="qxpp", bufs=3) as qxpp,
            tc.tile_pool(name="psq", bufs=1, space="PSUM") as psq,
            tc.tile_pool(name="psit", bufs=2, space="PSUM") as psit,
        ):
            g_sb = consts.tile([128, 5, P_PATCH], F32)
            nc.sync.dma_start(g_sb[:], gmat[:])

            pbase_sb = consts.tile([P_PATCH, 1], F32)
            nc.sync.dma_start(pbase_sb[:], pbase[:])
            id_sb = consts.tile([GH, GH], F32)
            nc.sync.dma_start(id_sb[:], id96[:])

            scr_act = scr.tile([128, D], F32)  # ACT throwaway output
            scr_dve = scr.tile([GH, D], F32)  # DVE throwaway output

            for s in range(SPC):
                x0 = s * SEQ
                o0 = s * OUT_SEQ

                # ---------------- low phase ----------------
                psum_q = psq.tile([P_PATCH, D], F32)
                for t in range(5):
                    rows = min(128, N_LOW - t * 128)
                    lt = lowp.tile([128, D], F32)
                    nc.sync.dma_start(
                        lt[:rows], x[x0 + t * 128 : x0 + t * 128 + rows, :]
                    )
                    ss = small.tile([128, 1], F32)
                    nc.scalar.activation(
                        scr_act[:rows], lt[:rows], AF.Square, accum_out=ss[:rows]
                    )
                    nrm = small.tile([128, 1], F32)
                    nc.scalar.activation(nrm[:rows], ss[:rows], AF.Sqrt)
                    rn = small.tile([128, 1], F32)
                    nc.vector.reciprocal(rn[:rows], nrm[:rows])
                    nc.vector.tensor_scalar_mul(lt[:rows], lt[:rows], rn[:rows])
                    for h in range(2):
                        nc.tensor.matmul(
                            psum_q[:, h * 512 : (h + 1) * 512],
                            lhsT=g_sb[:rows, t, :],
                            rhs=lt[:rows, h * 512 : (h + 1) * 512],
                            start=(t == 0),
                            stop=(t == 4),
                        )
                    nc.sync.dma_start(
                        out[o0 + t * 128 : o0 + t * 128 + rows, :], lt[:rows]
                    )
                q_sb = accs.tile([P_PATCH, D], F32)
                nc.scalar.activation(q_sb[:], psum_q[:], AF.Copy)

                # ---------------- high phase ----------------
                inner = accs.tile([GH, P_PATCH], F32)
                for p in range(P_PATCH):
                    r0 = x0 + N_LOW + p * GH
                    ht = highp.tile([GH, D], F32)
                    nc.sync.dma_start(ht[:], x[r0 : r0 + GH, :])
                    qrow = small.tile([1, D], F32)
                    nc.sync.dma_start(qrow[:], q_sb[p : p + 1, :])
                    pqx = qxpp.tile([GH, D], F32)
                    nc.gpsimd.partition_broadcast(pqx[:], qrow[:])
                    ssh = small.tile([GH, 1], F32)
                    nc.scalar.activation(
                        scr_act[:GH], ht[:], AF.Square, accum_out=ssh[:]
                    )
                    nrh = small.tile([GH, 1], F32)
                    nc.scalar.activation(nrh[:], ssh[:], AF.Sqrt)
                    rnh = small.tile([GH, 1], F32)
                    nc.vector.reciprocal(rnh[:], nrh[:])
                    # inner[:, p] = sum_d (h * rnorm) * qexp  — fused dot
                    nc.vector.scalar_tensor_tensor(
                        out=scr_dve[:],
                        in0=ht[:],
                        scalar=rnh[:],
                        in1=pqx[:],
                        op0=OP.mult,
                        op1=OP.mult,
                        accum_out=inner[:, p : p + 1],
                    )

                pit = psit.tile([P_PATCH, GH], F32)
                nc.tensor.transpose(pit[:], inner[:], id_sb[:])
                it_sb = accs.tile([P_PATCH, GH], F32)
                nc.scalar.activation(it_sb[:], pit[:], AF.Copy)

                mx8 = small.tile([P_PATCH, TOP_K], F32)
                nc.vector.max(out=mx8[:], in_=it_sb[:])
                ix8 = small.tile([P_PATCH, TOP_K], U32)
                nc.vector.max_index(out=ix8[:], in_max=mx8[:], in_values=it_sb[:])
                ixf = small.tile([P_PATCH, TOP_K], F32)
                nc.vector.tensor_copy(ixf[:], ix8[:])
                ixg = small.tile([P_PATCH, TOP_K], F32)
                nc.vector.tensor_scalar(
                    ixg[:],
                    ixf[:],
                    pbase_sb[:],
                    float(s * SEQ),
                    op0=OP.add,
                    op1=OP.add,
                )
                ixu = small.tile([P_PATCH, TOP_K], U32)
                nc.vector.tensor_copy(ixu[:], ixg[:])
                nc.sync.dma_start(
                    idxd[s].rearrange("(a b) c -> a (b c)", a=P_PATCH), ixu[:]
                )

                # ---------------- gather + renormalize ----------------
                for gi in range(2):
                    rows = 128 if gi == 0 else 64
                    base = gi * 128
                    ixcol = small.tile([128, 1], U32)
                    nc.sync.dma_start(ixcol[:rows], idxd[s, base : base + rows, :])
                    gt = gathp.tile([128, D], F32)
                    nc.gpsimd.indirect_dma_start(
                        out=gt[:rows],
                        out_offset=None,
                        in_=x[:],
                        in_offset=IndirectOffsetOnAxis(ap=ixcol[:rows], axis=0),
                    )
                    ssg = small.tile([128, 1], F32)
                    nc.scalar.activation(
                        scr_act[:rows], gt[:rows], AF.Square, accum_out=ssg[:rows]
                    )
                    nrg = small.tile([128, 1], F32)
                    nc.scalar.activation(nrg[:rows], ssg[:rows], AF.Sqrt)
                    rg = small.tile([128, 1], F32)
                    nc.vector.reciprocal(rg[:rows], nrg[:rows])
                    nc.vector.tensor_scalar_mul(gt[:rows], gt[:rows], rg[:rows])
                    nc.sync.dma_start(
                        out[o0 + N_LOW + base : o0 + N_LOW + base + rows, :],
                        gt[:rows],
                    )
    nc.finalize()
    if split_waits:
        _split_excess_waits(nc)
    return nc


_CACHED = {}


def _get_program():
    if "nc" not in _CACHED:
        _CACHED["nc"] = build_program()
    return _CACHED["nc"]


def kernel(x: np.ndarray) -> np.ndarray:
    assert x.shape == (BSZ, SEQ, D), x.shape
    x = np.ascontiguousarray(x, dtype=np.float32)
    consts = host_constants()
    shards = x.reshape(N_CORES, SPC * SEQ, D)
    in_maps = [dict(consts, x=shards[i]) for i in range(N_CORES)]
    nc = _get_program()
    res = run_bass_kernel_spmd(nc, in_maps, core_ids=list(range(N_CORES)))
    outs = [res.results[i]["out"].reshape(SPC, OUT_SEQ, D) for i in range(N_CORES)]
    return np.concatenate(outs, axis=0).astype(np.float32)
